# revision 1
# baseline (speedup 1.0000x reference)
"""AttentionBlock (GroupNorm + single-head self-attention + proj + residual)
on 8 Trainium2 NeuronCores, data-parallel over the batch dimension.

Reference computation (per batch b):
    h  = group_norm(x, 32 groups, eps=1e-5) * gn_w + gn_b
    qkv = qkv_w @ h + qkv_b            (1x1 conv == per-pixel linear)
    S[i,j] = (q[:,i] . k[:,j]) * C**-0.5
    P = softmax_j(S)
    out = proj_w @ (P @ v) + proj_b
    y = x + out

Layout strategy per core (4 batches, all on-chip after the x load):
    h, q, k      : [C, N]  (channels on partitions)   C=512 -> 4 chunks of 128
    vT           : [N, C]  (pixels on partitions)     N=1024 -> 8 chunks
    expST        : [N_j, N_i] = exp(S^T)              softmax denominators via
                   ones-vector matmul (reduces partition dim j), broadcast back
                   with a K=1 outer-product matmul, divide fused into the
                   PSUM->SBUF evacuation of the attention output.
    attn         : [C, N] directly (lhsT=vT chunks)   -> proj needs no transpose
"""

import os

import numpy as np

import concourse.bacc as bacc
import concourse.bass as bass
import concourse.mybir as mybir
import concourse.tile as tile
from concourse.bass_utils import run_bass_kernel_spmd

P = 128
B, C, H, W = 32, 512, 32, 32
N = H * W                      # 1024 pixels
NCORES = 8
BPC = B // NCORES              # 4 batches per core
GROUPS = 32
GSIZE = C // GROUPS            # 16 channels per group
EPS = 1e-5
ATTN_SCALE = float(C) ** -0.5

CK = C // P                    # 4 channel chunks
NK = N // P                    # 8 pixel chunks
FD = 512                       # matmul moving free dim (fp32 max, 1 PSUM bank)
NI = N // FD                   # 2 free-dim chunks over pixels

F32 = mybir.dt.float32
MM_DT = mybir.dt.float32r if os.environ.get("ATTN_MM_DT", "f32r") == "f32r" \
    else mybir.dt.float32


def build_nc(mm_dt=None, n_loop: int = 1, psum_bufs: int = 6, dve_affine: bool = False,
             skip_gn: bool = False, unit_softmax: bool = False, stagger: bool = False,
             skip_out: bool = False, x_bufs: int = 2, psaux_bufs: int = 2):
    if mm_dt is None:
        mm_dt = MM_DT
    nc = bacc.Bacc()

    x_d = nc.declare_dram_parameter("x", [BPC, C, N], F32, isOutput=False)
    qkvwT_d = nc.declare_dram_parameter("qkvwT", [C, 3 * C], mm_dt, isOutput=False)
    projwT_d = nc.declare_dram_parameter("projwT", [C, C], mm_dt, isOutput=False)
    qkvb_d = nc.declare_dram_parameter("qkvb", [3 * C], F32, isOutput=False)
    projb_d = nc.declare_dram_parameter("projb", [C], F32, isOutput=False)
    gnw_d = nc.declare_dram_parameter("gnw", [C], F32, isOutput=False)
    gnb_d = nc.declare_dram_parameter("gnb", [C], F32, isOutput=False)
    gavg_d = nc.declare_dram_parameter("gavg", [P, P], F32, isOutput=False)
    ones128_d = nc.declare_dram_parameter("ones128", [P, P], mm_dt, isOutput=False)
    out_d = nc.declare_dram_parameter("out", [BPC, C, N], F32, isOutput=True)

    NG = P // GSIZE            # 8 groups per 128-channel chunk

    from contextlib import ExitStack
    with tile.TileContext(nc) as tc, ExitStack() as ctx:
        consts = ctx.enter_context(tc.tile_pool(name="consts", bufs=1))
        big = ctx.enter_context(tc.tile_pool(name="big", bufs=1))
        xpool = ctx.enter_context(tc.tile_pool(name="xpool", bufs=x_bufs))
        small = ctx.enter_context(tc.tile_pool(name="small", bufs=2))
        psum = ctx.enter_context(tc.tile_pool(name="psum", bufs=psum_bufs, space="PSUM"))
        psaux = ctx.enter_context(tc.tile_pool(name="psaux", bufs=psaux_bufs, space="PSUM"))

        # ---- batch-0 activations first: GN depends only on x, so let its
        # DMA land before the (large) weight loads share queue bandwidth ----
        x0_t = None
        if n_loop == 1:
            x0_t = []
            for kk in range(CK):
                t = xpool.tile([P, N], F32, name=f"x{kk}")
                nc.sync.dma_start(out=t, in_=x_d[0, kk * P:(kk + 1) * P, :])
                x0_t.append(t)

        # ---- constants (loaded once) ----
        wqkv = []
        for kk in range(CK):
            t = consts.tile([P, 3 * C], mm_dt, name=f"wqkv{kk}")
            nc.sync.dma_start(out=t, in_=qkvwT_d[kk * P:(kk + 1) * P, :])
            wqkv.append(t)
        wproj = []
        for kk in range(CK):
            t = consts.tile([P, C], mm_dt, name=f"wproj{kk}")
            nc.sync.dma_start(out=t, in_=projwT_d[kk * P:(kk + 1) * P, :])
            wproj.append(t)
        gavg = consts.tile([P, P], F32, name="gavg")
        nc.sync.dma_start(out=gavg, in_=gavg_d[:, :])
        ones128 = consts.tile([P, P], mm_dt, name="ones128")
        nc.sync.dma_start(out=ones128, in_=ones128_d[:, :])
        eps_t = consts.tile([P, 1], F32, name="eps")
        nc.vector.memset(eps_t, EPS)
        gnw = consts.tile([P, CK], F32, name="gnw")
        nc.sync.dma_start(out=gnw, in_=gnw_d[:].rearrange("(t c) -> c t", t=CK))
        gnb = consts.tile([P, CK], F32, name="gnb")
        nc.sync.dma_start(out=gnb, in_=gnb_d[:].rearrange("(t c) -> c t", t=CK))
        qb = consts.tile([P, 3 * CK], F32, name="qb")
        nc.sync.dma_start(out=qb, in_=qkvb_d[:].rearrange("(m c) -> c m", m=3 * CK))
        pb = consts.tile([P, CK], F32, name="pb")
        nc.sync.dma_start(out=pb, in_=projb_d[:].rearrange("(t c) -> c t", t=CK))
        # v-bias broadcast along partitions: [1,512] -> [128,512]
        vbias = consts.tile([P, C], F32, name="vbias")
        vb_src = qkvb_d[2 * C:3 * C]
        nc.sync.dma_start(
            out=vbias,
            in_=bass.AP(tensor=vb_src.tensor, offset=vb_src.offset,
                        ap=[[0, P]] + list(vb_src.ap)),
        )

        def mm(ps, lhsT, rhs, start, stop):
            nc.tensor.matmul(ps, lhsT=lhsT, rhs=rhs, start=start, stop=stop)

        def stage_a(b):
            nonlocal x0_t
            # ---- load x (kept for the residual) ----
            if b == 0 and x0_t is not None:
                x_t = list(x0_t)
            else:
                x_t = []
                for kk in range(CK):
                    t = xpool.tile([P, N], F32, name=f"x{kk}")
                    nc.sync.dma_start(out=t, in_=x_d[b, kk * P:(kk + 1) * P, :])
                    x_t.append(t)

            # ---- GroupNorm statistics ----
            if skip_gn:
                h_t = []
                for kk in range(CK):
                    t = big.tile([P, N], mm_dt, name=f"h{kk}")
                    nc.scalar.activation(out=t, in_=x_t[kk],
                                         func=mybir.ActivationFunctionType.Copy)
                    h_t.append(t)
            else:
                # per-channel mean/var via bn_stats, then 16-channel group
                # reduction with a one-hot matmul.
                # per-channel mean/E[x^2] via bn_stats, then one
                # group-averaging matmul (gavg = block-diag 1/16) gives the
                # per-channel group stats directly (reduce+broadcast fused)
                ps_pc = psaux.tile([P, 2 * CK], F32, name="aux")
                for kk in range(CK):
                    bn6 = small.tile([P, 2, 6], F32, name="bn6")
                    nc.vector.bn_stats(out=bn6[:, 0, :], in_=x_t[kk][:, 0:FD])
                    nc.vector.bn_stats(out=bn6[:, 1, :], in_=x_t[kk][:, FD:N])
                    mv = small.tile([P, 2], F32, name=f"mv{kk}")
                    nc.vector.bn_aggr(out=mv, in_=bn6)
                    # mv[:,1] <- var + mean^2  (= E[x^2] per channel)
                    m2 = small.tile([P, 1], F32, name="m2")
                    nc.vector.tensor_mul(m2, mv[:, 0:1], mv[:, 0:1])
                    nc.vector.tensor_add(mv[:, 1:2], mv[:, 1:2], m2)
                    nc.tensor.matmul(ps_pc[:, 2 * kk:2 * kk + 2], lhsT=gavg,
                                     rhs=mv, start=True, stop=True)
                pc = small.tile([P, CK, 2], F32, name="pc")
                nc.scalar.activation(out=pc, in_=ps_pc.rearrange("c (k two) -> c k two", two=2),
                                     func=mybir.ActivationFunctionType.Copy)
                # pc[:,:,0]=group mean, pc[:,:,1]=group E[x^2] per channel
                gm2 = small.tile([P, CK], F32, name="gm2")
                nc.vector.tensor_mul(gm2, pc[:, :, 0], pc[:, :, 0])
                nc.vector.tensor_sub(pc[:, :, 1], pc[:, :, 1], gm2)
                nc.scalar.activation(out=pc[:, :, 1], in_=pc[:, :, 1],
                                     func=mybir.ActivationFunctionType.Sqrt,
                                     bias=eps_t, scale=1.0)
                nc.vector.reciprocal(out=pc[:, :, 1], in_=pc[:, :, 1])
                # fold gn affine: scale_c = rstd*gn_w ; bias_c = gn_b - mean*scale_c
                sc = small.tile([P, CK], F32, name="sc")
                nc.vector.tensor_mul(sc, pc[:, :, 1], gnw)
                bi = small.tile([P, CK], F32, name="bi")
                nc.vector.tensor_mul(bi, pc[:, :, 0], sc)
                nc.vector.tensor_sub(bi, gnb, bi)

                # ---- normalize: h = x*scale + bias ----
                h_t = []
                for kk in range(CK):
                    t = big.tile([P, N], mm_dt, name=f"h{kk}")
                    if dve_affine:
                        nc.vector.tensor_scalar(out=t, in0=x_t[kk],
                                                scalar1=sc[:, kk:kk + 1],
                                                scalar2=bi[:, kk:kk + 1],
                                                op0=mybir.AluOpType.mult,
                                                op1=mybir.AluOpType.add)
                    else:
                        nc.scalar.activation(out=t, in_=x_t[kk],
                                             func=mybir.ActivationFunctionType.Identity,
                                             scale=sc[:, kk:kk + 1],
                                             bias=bi[:, kk:kk + 1])
                    h_t.append(t)

            return x_t, h_t

        def stage_b1(b, x_t, h_t):
            # ---- q, k: [C, N] ----
            q_t = [big.tile([P, N], mm_dt, name=f"q{m}") for m in range(CK)]
            k_t = [big.tile([P, N], mm_dt, name=f"k{m}") for m in range(CK)]
            for which, dst in ((0, q_t), (1, k_t)):
                for m in range(CK):
                    wcol = which * C + m * P
                    for ni in range(NI):
                        ps = psum.tile([P, FD], F32, name="mm")
                        for kk in range(CK):
                            mm(ps, wqkv[kk][:, wcol:wcol + P],
                               h_t[kk][:, ni * FD:(ni + 1) * FD],
                               kk == 0, kk == CK - 1)
                        nc.scalar.activation(
                            out=dst[m][:, ni * FD:(ni + 1) * FD], in_=ps,
                            func=mybir.ActivationFunctionType.Identity,
                            bias=qb[:, which * CK + m:which * CK + m + 1])

            # ---- vT: [N, C] ----
            v_t = [big.tile([P, C], mm_dt, name=f"v{mn}") for mn in range(NK)]
            for mn in range(NK):
                ps = psum.tile([P, FD], F32, name="mm")
                for kk in range(CK):
                    mm(ps, h_t[kk][:, mn * P:(mn + 1) * P],
                       wqkv[kk][:, 2 * C:3 * C], kk == 0, kk == CK - 1)
                nc.vector.tensor_add(v_t[mn], ps, vbias)

            return q_t, k_t, v_t

        def stage_s(b, q_t, k_t, v_t):
            # ---- S^T & exp: expST[j, i] = exp(scale * sum_c k[c,j] q[c,i]) ----
            e_t = [big.tile([P, N], mm_dt, name=f"e{mj}") for mj in range(NK)]
            psr = None
            if not unit_softmax:
                psr = [psaux.tile([P, FD], F32, name="aux") for _ in range(NI)]
            for ni in range(NI):
                for mj in range(NK):
                    ps = psum.tile([P, FD], F32, name="mm")
                    for kk in range(CK):
                        mm(ps, k_t[kk][:, mj * P:(mj + 1) * P],
                           q_t[kk][:, ni * FD:(ni + 1) * FD],
                           kk == 0, kk == CK - 1)
                    nc.scalar.activation(
                        out=e_t[mj][:, ni * FD:(ni + 1) * FD], in_=ps,
                        func=mybir.ActivationFunctionType.Exp,
                        scale=ATTN_SCALE)
                # batched after the half-stage: only the last chunk's exp
                # is still in flight, so PE (in-order) barely stalls
                if psr is not None:
                    for mj in range(NK):
                        mm(psr[ni], ones128,
                           e_t[mj][:, ni * FD:(ni + 1) * FD],
                           mj == 0, mj == NK - 1)

            return e_t, psr

        def stage_b2(b, x_t, q_t, k_t, v_t, e_t, psr):
            # ---- softmax denominators ----
            if unit_softmax:
                invb = big.tile([P, N], F32, name="invb")
                nc.vector.memset(invb, 1.0)
            else:
                # psr already holds the rowsum on every partition
                invb = big.tile([P, N], F32, name="invb")
                for ni in range(NI):
                    nc.vector.reciprocal(out=invb[:, ni * FD:(ni + 1) * FD],
                                         in_=psr[ni])

            # ---- attn = (P @ v) in [C, N]: lhsT=vT chunk, rhs=expST ----
            a_t = [big.tile([P, N], mm_dt, name=f"k{mc}") for mc in range(CK)]
            for ni in range(NI):
                for mc in range(CK):
                    ps = psum.tile([P, FD], F32, name="mm")
                    for jj in range(NK):
                        mm(ps, v_t[jj][:, mc * P:(mc + 1) * P],
                           e_t[jj][:, ni * FD:(ni + 1) * FD],
                           jj == 0, jj == NK - 1)
                    nc.vector.tensor_mul(
                        a_t[mc][:, ni * FD:(ni + 1) * FD], ps,
                        invb[:, ni * FD:(ni + 1) * FD])

            # ---- x <- x + proj_b (residual base) ----
            for kk in range(CK):
                if dve_affine:
                    nc.vector.tensor_scalar_add(out=x_t[kk], in0=x_t[kk],
                                                scalar1=pb[:, kk:kk + 1])
                else:
                    nc.scalar.activation(out=x_t[kk], in_=x_t[kk],
                                         func=mybir.ActivationFunctionType.Identity,
                                         bias=pb[:, kk:kk + 1])

            # ---- proj + residual + store ----
            o_t = [big.tile([P, N], F32, name=f"q{mo}") for mo in range(CK)]
            for ni in range(NI):
                for mo in range(CK):
                    ps = psum.tile([P, FD], F32, name="mm")
                    for kk in range(CK):
                        mm(ps, wproj[kk][:, mo * P:(mo + 1) * P],
                           a_t[kk][:, ni * FD:(ni + 1) * FD],
                           kk == 0, kk == CK - 1)
                    nc.vector.tensor_add(o_t[mo][:, ni * FD:(ni + 1) * FD], ps,
                                         x_t[mo][:, ni * FD:(ni + 1) * FD])
                    if ni == NI - 1 and (not skip_out or (b == 0 and mo == 0)):
                        nc.sync.dma_start(
                            out=out_d[b, mo * P:(mo + 1) * P, :], in_=o_t[mo])



        def batch_body():
            st = stage_a(0)
            for b in range(BPC):
                x_t, h_t = st
                qkv = stage_b1(b, x_t, h_t)
                e_t, psr = stage_s(b, *qkv)
                if b + 1 < BPC:
                    st = stage_a(b + 1)
                stage_b2(b, x_t, qkv[0], qkv[1], qkv[2], e_t, psr)
        if n_loop == 1:
            batch_body()
        else:
            with tc.For_i(0, n_loop, staggered_reset=stagger,
                          hint_engines=(mybir.EngineType.PE,)):
                batch_body()

    nc.compile()
    return nc


def _aux_arrays(gn_w, gn_b, qkv_w, qkv_b, proj_w, proj_b):
    grp = np.arange(P) // GSIZE
    gavg = (grp[:, None] == grp[None, :]).astype(np.float32) / GSIZE
    return {
        "qkvwT": np.ascontiguousarray(qkv_w.T.astype(np.float32)),
        "projwT": np.ascontiguousarray(proj_w.T.astype(np.float32)),
        "qkvb": np.ascontiguousarray(qkv_b.astype(np.float32)),
        "projb": np.ascontiguousarray(proj_b.astype(np.float32)),
        "gnw": np.ascontiguousarray(gn_w.astype(np.float32)),
        "gnb": np.ascontiguousarray(gn_b.astype(np.float32)),
        "gavg": gavg,
        "ones128": np.ones((P, P), np.float32),
    }


def make_in_maps(x, gn_w, gn_b, qkv_w, qkv_b, proj_w, proj_b):
    aux = _aux_arrays(gn_w, gn_b, qkv_w, qkv_b, proj_w, proj_b)
    x = np.asarray(x, np.float32).reshape(B, C, N)
    in_maps = []
    for c in range(NCORES):
        m = {"x": np.ascontiguousarray(x[c * BPC:(c + 1) * BPC])}
        m.update(aux)
        in_maps.append(m)
    return in_maps


_NC_CACHE = {}


def _get_nc(key=("default", 1)):
    if key not in _NC_CACHE:
        mm_dt = MM_DT if key[0] == "default" else key[0]
        _NC_CACHE[key] = build_nc(mm_dt=mm_dt, n_loop=key[1])
    return _NC_CACHE[key]


def kernel(x, gn_w, gn_b, qkv_w, qkv_b, proj_w, proj_b):
    nc = _get_nc()
    in_maps = make_in_maps(x, gn_w, gn_b, qkv_w, qkv_b, proj_w, proj_b)
    res = run_bass_kernel_spmd(nc, in_maps, list(range(NCORES)))
    out = np.concatenate([res.results[c]["out"] for c in range(NCORES)], axis=0)
    return out.reshape(B, C, H, W).astype(np.float32)


if __name__ == "__main__":
    rng = np.random.default_rng(0)
    x = rng.standard_normal((B, C, H, W)).astype(np.float32)
    out = kernel(
        x,
        np.ones(C, np.float32), np.zeros(C, np.float32),
        (rng.standard_normal((3 * C, C)) * C ** -0.5).astype(np.float32),
        np.zeros(3 * C, np.float32),
        (rng.standard_normal((C, C)) * C ** -0.5).astype(np.float32),
        np.zeros(C, np.float32),
    )
    print(out.shape, out.dtype)

